# revision 24
# baseline (speedup 1.0000x reference)
"""Causal self-attention (B=4, S=2048, D=1024, H=16) on 8 TRN2 NeuronCores.

Sharding: batch 4-way x head-group 2-way. Core c handles batch c//2 and
heads (c%2)*8 .. (c%2)*8+8. Each core computes its QKV projection slice,
per-head causal attention, and a partial output projection (W_out rows of
its heads); the host sums the two head-group partials per batch.

v2 design (vs v1 baseline):
- bf16 storage everywhere on SBUF (halves DMA + enables DVE 2x + FWL).
- qs-major software pipeline: projection (P1) for quarter qs+1 and
  output projection (P3) for quarter qs-1 are emitted as "filler" PE
  work interleaved into the Act-bound attention groups of quarter qs,
  so the TensorEngine never idles long enough for HAM to re-throttle.
- softmax normalize off the critical path: PSUM attention outputs are
  copied to SBUF immediately (frees the PSUM bank for the next head
  pair), denominators batched into one DVE reciprocal per quarter,
  broadcast on gpsimd, final scale on DVE.
- exp over a whole group (2 k-chunks x 2 heads = [128,4,512]) in one
  Act instruction to amortize the read-write bubble.
"""

import os
import sys

for _p in ("/opt/trn_rl_repo", "/root/.axon_site/_ro/trn_rl_repo"):
    if os.path.isdir(_p) and _p not in sys.path:
        sys.path.insert(0, _p)

import numpy as np
import ml_dtypes

import concourse.bass as bass  # noqa: E402
import concourse.tile as tile  # noqa: E402
from concourse import bacc, mybir  # noqa: E402
from concourse.bass_utils import run_bass_kernel_spmd  # noqa: E402

F32 = mybir.dt.float32
R32 = mybir.dt.float32r
BF16 = mybir.dt.bfloat16
BF16_NP = ml_dtypes.bfloat16

B = 4
S = 2048
D = 1024
H = 16
HD = 64  # head dim
HLOC = 8  # heads per core
SCALE = HD ** -0.5

C = D // 128  # contraction chunks (8)
NQS = S // 512  # token quarters (4)
NST = S // 128  # 128-token tiles (16)
NT = HLOC * HD // 128  # head pairs per core (4)
NFC = NT


def _build_nc(use_bias: bool, repeat: int = 1):
    from contextlib import ExitStack

    nc = bacc.Bacc(
        "TRN2",
        target_bir_lowering=False,
        debug=False,
        enable_asserts=True,
        num_devices=8,
    )

    dw = D + 1 if use_bias else D
    cw = C + 1 if use_bias else C
    xT = nc.dram_tensor("xT", [dw, S], BF16, kind="ExternalInput")
    wq = nc.dram_tensor("wq", [dw, 512], BF16, kind="ExternalInput")
    wk = nc.dram_tensor("wk", [dw, 512], BF16, kind="ExternalInput")
    wv = nc.dram_tensor("wv", [dw, 512], BF16, kind="ExternalInput")
    wo = nc.dram_tensor("wo", [512, D], BF16, kind="ExternalInput")
    mask = nc.dram_tensor("mask", [128, 2, 128], BF16, kind="ExternalInput")
    y = nc.dram_tensor("y", [S, D], F32, kind="ExternalOutput")

    with tile.TileContext(nc) as tc, ExitStack() as es:
        if repeat > 1:
            es.enter_context(
                tc.For_i(
                    0,
                    repeat,
                    1,
                    hint_engines=(
                        mybir.EngineType.PE,
                        mybir.EngineType.Activation,
                        mybir.EngineType.DVE,
                    ),
                )
            )
        persist = es.enter_context(tc.tile_pool(name="persist", bufs=1))
        trans = es.enter_context(tc.tile_pool(name="trans", bufs=1))
        psum = es.enter_context(tc.tile_pool(name="psum", bufs=1, space="PSUM"))

        QT_sb = persist.tile([128, NT, S], BF16)
        KT_sb = persist.tile([128, NT, S], BF16)
        V_sb = persist.tile([128, NST, HLOC, HD + 1], BF16)
        vals_sb = persist.tile([128, NFC, S], BF16)
        mask_sb = persist.tile([128, 2, 128], BF16)
        ones_sb = persist.tile([128, HD], R32)
        wq_sb = persist.tile([128, cw, 512], BF16)
        wk_sb = persist.tile([128, cw, 512], BF16)
        wv_sb = persist.tile([128, cw, 512], BF16)
        wo_sb = persist.tile([128, NFC, D], BF16)

        xqs = []  # per-quarter x tiles, allocated lazily with bufs=2

        def dma_xq(qs):
            sq = slice(qs * 512, (qs + 1) * 512)
            xq = trans.tile([128, cw, 512], BF16, tag="xq", bufs=2, name=f"xq{qs}")
            # split into halves so two DMA queues run in parallel
            for h in range(2):
                nc.sync.dma_start(
                    xq[:, 4 * h : 4 * h + 4, :],
                    xT[512 * h : 512 * h + 512, sq].rearrange(
                        "(c p) s -> p c s", p=128
                    ),
                )
            if use_bias:
                nc.vector.memset(xq[0:1, C, :], 1.0)
            while len(xqs) <= qs:
                xqs.append(None)
            xqs[qs] = xq

        def p1_units(qs):
            """Projection work for quarter qs: 12 units of 8 matmuls + copy."""
            sq = slice(qs * 512, (qs + 1) * 512)

            def qk_unit(wsb, dst, t):
                def run():
                    xq = xqs[qs]
                    qp = psum.tile([128, 512], F32, tag="misc", bufs=2, name="mm_ps")
                    for c in range(C):
                        nc.tensor.matmul(
                            qp[:],
                            lhsT=wsb[:, c, t * 128 : (t + 1) * 128],
                            rhs=xq[:, c, :],
                            start=(c == 0),
                            stop=(c == C - 1 and not use_bias),
                        )
                    if use_bias:
                        nc.tensor.matmul(
                            qp[:],
                            lhsT=wsb[0:1, C, t * 128 : (t + 1) * 128],
                            rhs=xq[0:1, C, :],
                            start=False,
                            stop=True,
                        )
                    nc.vector.tensor_copy(dst[:, t, sq], qp[:])

                return run

            def v_unit(sst):
                def run():
                    xq = xqs[qs]
                    st = qs * 4 + sst
                    sl = slice(sst * 128, (sst + 1) * 128)
                    vp = psum.tile([128, 512], F32, tag="misc", bufs=2, name="v_ps")
                    for c in range(C):
                        nc.tensor.matmul(
                            vp[:],
                            lhsT=xq[:, c, sl],
                            rhs=wv_sb[:, c, :],
                            start=(c == 0),
                            stop=(c == C - 1 and not use_bias),
                        )
                    if use_bias:
                        nc.tensor.matmul(
                            vp[:],
                            lhsT=xq[0:1, C, sl],
                            rhs=wv_sb[0:1, C, :],
                            start=False,
                            stop=True,
                        )
                    nc.vector.tensor_copy(
                        V_sb[:, st, :, 0:HD],
                        vp.rearrange("p (h e) -> p h e", h=HLOC),
                    )

                return run

            units = []
            for t in range(NT):
                units.append(qk_unit(wq_sb, QT_sb, t))
                units.append(qk_unit(wk_sb, KT_sb, t))
            for sst in range(4):
                units.append(v_unit(sst))
            return units

        def p3_units(qs):
            """Output projection for quarter qs: 8 units of 4 matmuls."""

            def unit(sst, nh):
                def run():
                    st = qs * 4 + sst
                    sl = slice(st * 128, (st + 1) * 128)
                    nsl = slice(nh * 512, (nh + 1) * 512)
                    yp = psum.tile([128, 512], F32, tag="misc", bufs=2, name="y_ps")
                    for fc in range(NFC):
                        nc.tensor.matmul(
                            yp[:],
                            lhsT=vals_sb[:, fc, sl],
                            rhs=wo_sb[:, fc, nsl],
                            start=(fc == 0),
                            stop=(fc == NFC - 1),
                        )
                    yo = trans.tile([128, 512], F32, tag="yo", bufs=3, name="yo")
                    nc.vector.tensor_copy(yo[:], yp[:])
                    nc.gpsimd.dma_start(y[sl, nsl], yo[:])

                return run

            return [unit(sst, nh) for sst in range(4) for nh in range(2)]

        def norm_units(qs, o_sbs, rc_all, ts):
            """Normalize head pairs `ts` of quarter qs into vals_sb."""
            sq = slice(qs * 512, (qs + 1) * 512)

            def unit(t, p):
                def run():
                    tp = t * 2 + p
                    rp, rcol = 32 * (tp % 4), tp // 4
                    rc_ps = psum.tile([64, 512], F32, tag="misc", bufs=2,
                                      name="rc_ps")
                    nc.tensor.matmul(
                        rc_ps[:],
                        lhsT=ones_sb[rp : rp + 1, :],
                        rhs=rc_all[rp : rp + 1, rcol, :],
                        start=True,
                        stop=True,
                        tile_position=(rp, 0),
                    )
                    nc.vector.tensor_mul(
                        vals_sb[p * HD : (p + 1) * HD, t, sq],
                        o_sbs[tp][0:HD, :],
                        rc_ps[:],
                    )

                return run

            return [unit(t, p) for t in ts for p in range(2)]

        queue = []
        p1_pending = [0] * NQS

        def pull(n=1):
            for _ in range(n):
                if queue:
                    queue.pop(0)()

        def mark_done(qs, fn):
            def run():
                fn()
                p1_pending[qs] -= 1

            return run

        # DMA order: quarter-0 x and wq first so the first projection
        # matmuls start as early as possible.
        dma_xq(0)
        for wsb, wdr in ((wq_sb, wq), (wk_sb, wk), (wv_sb, wv)):
            for h in range(2):
                nc.sync.dma_start(
                    wsb[:, 4 * h : 4 * h + 4, :],
                    wdr[512 * h : 512 * h + 512, :].rearrange(
                        "(c p) f -> p c f", p=128
                    ),
                )
            if use_bias:
                nc.sync.dma_start(wsb[0:1, C, :], wdr[D : D + 1, :])
        nc.sync.dma_start(wo_sb[:], wo.rearrange("(c p) n -> p c n", p=128))
        nc.sync.dma_start(mask_sb[:], mask[:])
        # ones column feeds the softmax denominator through the PV matmul
        nc.vector.memset(V_sb[:, :, :, HD : HD + 1], 1.0)
        # ones rows for PE-based partition broadcast of 1/denominator
        # (gpsimd partition_broadcast reads the wrong partition on HW for
        # non-zero input bases; a K=1 ones-matmul is a well-defined bcast)
        nc.vector.memset(ones_sb.bitcast(F32), 1.0)

        # quarter 0 projection up front (nothing to interleave with yet)
        for u in p1_units(0):
            u()

        for qs in range(NQS):
            # correctness barrier: this quarter's Q/K/V must be fully
            # emitted (they were queued as fillers last iteration).
            while p1_pending[qs] > 0:
                pull(1)
            if qs + 1 < NQS:
                dma_xq(qs + 1)
                units = [mark_done(qs + 1, u) for u in p1_units(qs + 1)]
                p1_pending[qs + 1] = len(units)
                queue.extend(units)
            sq = slice(qs * 512, (qs + 1) * 512)
            # denominators parked at partitions {0,32,64,96} x 2 columns
            # (SBUF APs must start at a multiple-of-32 partition); unused
            # lanes memset to 1.0 so the batched reciprocal stays finite.
            dstack = trans.tile([128, 2, 512], F32, tag="dstack", bufs=2,
                                name="dstack")
            nc.vector.memset(dstack[:], 1.0)
            rc_all = trans.tile([128, 2, 512], R32, tag="rc_all", bufs=2,
                                name="rc_all")
            o_sbs = {}
            # diagonal chunks first so the first PV write covers [0:512]
            js = list(range(4 * qs, 4 * qs + 4)) + list(range(0, 4 * qs))
            npos = len(js)
            for t in range(NT):
                outs = [
                    psum.tile([HD + 1, 512], F32, tag=f"out{p}", bufs=1,
                              name=f"out{p}")
                    for p in range(2)
                ]
                for pos, j in enumerate(js):
                    # diagonal chunks: q columns below the block diagonal
                    # are fully masked -> skip them in QKT/exp/PV entirely
                    qlo = pos * 128 if pos < 4 else 0
                    sc = psum.tile([128, 2, 512], F32, tag="sc", bufs=2, name="sc")
                    for p in range(2):
                        po = p * HD
                        nc.tensor.matmul(
                            sc[:, p, qlo:512],
                            lhsT=KT_sb[po : po + HD, t, j * 128 : (j + 1) * 128],
                            rhs=QT_sb[po : po + HD, t,
                                      qs * 512 + qlo : (qs + 1) * 512],
                            start=True,
                            stop=True,
                            tile_position=(po, 0),
                        )
                    ex = trans.tile([128, 2, 512], BF16, tag="ex", bufs=3, name="ex")
                    nc.scalar.activation(
                        ex[:, :, qlo:512],
                        sc[:, :, qlo:512],
                        mybir.ActivationFunctionType.Exp,
                        scale=SCALE,
                    )
                    if pos < 4:  # mask the 128-wide mixed band (both heads)
                        nc.vector.tensor_mul(
                            ex[:, :, qlo : qlo + 128],
                            ex[:, :, qlo : qlo + 128],
                            mask_sb[:],
                        )
                    for p in range(2):
                        nc.tensor.matmul(
                            outs[p][:, qlo:512],
                            lhsT=V_sb[:, j, 2 * t + p, :],
                            rhs=ex[:, p, qlo:512],
                            start=(pos == 0),
                            stop=(pos == npos - 1),
                        )
                    if pos % 2 == 1:
                        pull(1)
                # drain PSUM fast: copy attention outputs + denominators to
                # SBUF so the next head pair's PV can start immediately.
                for p in range(2):
                    tp = t * 2 + p
                    rp, rcol = 32 * (tp % 4), tp // 4
                    o_sb = trans.tile([HD + 1, 512], F32, tag="o_sb", bufs=10,
                                      name="o_sb")
                    nc.vector.tensor_copy(o_sb[:], outs[p][:])
                    nc.vector.tensor_copy(
                        dstack[rp : rp + 1, rcol, :], o_sb[HD : HD + 1, :]
                    )
                    o_sbs[tp] = o_sb
                if t == 1 or t == 3:
                    # half the denominators are complete: reciprocal that
                    # column now so norm units never stall the PE.
                    col = t // 2
                    with nc.allow_low_precision(reason="1/denom fp32r"):
                        nc.vector.reciprocal(
                            rc_all[:, col, :], dstack[:, col, :]
                        )
                    queue.extend(norm_units(qs, o_sbs, rc_all, (t - 1, t)))
            queue.extend(p3_units(qs))

        while queue:
            pull(1)

    nc.finalize()
    return nc


_NC_CACHE = {}


def _get_nc(use_bias: bool, repeat: int = 1):
    key = (use_bias, repeat)
    if key not in _NC_CACHE:
        _NC_CACHE[key] = _build_nc(use_bias, repeat)
    return _NC_CACHE[key]


def _make_mask() -> np.ndarray:
    # keep[k_local, q_local] = q_local >= k_local; duplicated for both
    # heads of a pair: [128, 2, 128]
    kl = np.arange(128)[:, None]
    ql = np.arange(128)[None, :]
    m = (ql >= kl).astype(BF16_NP)
    return np.ascontiguousarray(np.broadcast_to(m[:, None, :], (128, 2, 128)))


def make_in_maps(x, W_qkv, b_qkv, W_out):
    use_bias = bool(np.any(b_qkv))
    mask = _make_mask()
    in_maps = []
    for core in range(8):
        b = core // 2
        hg = core % 2
        xt = np.ascontiguousarray(x[b].T)  # [D, S]
        q_cols = slice(hg * 512, (hg + 1) * 512)
        k_cols = slice(D + hg * 512, D + (hg + 1) * 512)
        v_cols = slice(2 * D + hg * 512, 2 * D + (hg + 1) * 512)
        wq_s = np.ascontiguousarray(W_qkv[:, q_cols])
        wk_s = np.ascontiguousarray(W_qkv[:, k_cols])
        wv_s = np.ascontiguousarray(W_qkv[:, v_cols])
        if use_bias:
            xt = np.concatenate([xt, np.ones((1, S), np.float32)], axis=0)
            wq_s = np.concatenate([wq_s, b_qkv[None, hg * 512 : (hg + 1) * 512]], axis=0)
            wk_s = np.concatenate(
                [wk_s, b_qkv[None, D + hg * 512 : D + (hg + 1) * 512]], axis=0
            )
            wv_s = np.concatenate(
                [wv_s, b_qkv[None, 2 * D + hg * 512 : 2 * D + (hg + 1) * 512]], axis=0
            )
        wo_s = np.ascontiguousarray(W_out[hg * 512 : (hg + 1) * 512, :])
        in_maps.append(
            {
                "xT": np.ascontiguousarray(xt).astype(BF16_NP),
                "wq": wq_s.astype(BF16_NP),
                "wk": wk_s.astype(BF16_NP),
                "wv": wv_s.astype(BF16_NP),
                "wo": wo_s.astype(BF16_NP),
                "mask": mask,
            }
        )
    return in_maps, use_bias


def gather_output(results, b_out):
    y = np.empty((B, S, D), dtype=np.float32)
    for b in range(B):
        y[b] = results[2 * b]["y"] + results[2 * b + 1]["y"]
    if b_out is not None and np.any(b_out):
        y += b_out[None, None, :].astype(np.float32)
    return y


def kernel(x, W_qkv, b_qkv, W_out, b_out):
    x = np.asarray(x, dtype=np.float32)
    W_qkv = np.asarray(W_qkv, dtype=np.float32)
    b_qkv = np.asarray(b_qkv, dtype=np.float32)
    W_out = np.asarray(W_out, dtype=np.float32)
    b_out = np.asarray(b_out, dtype=np.float32)
    in_maps, use_bias = make_in_maps(x, W_qkv, b_qkv, W_out)
    nc = _get_nc(use_bias)
    res = run_bass_kernel_spmd(nc, in_maps, core_ids=list(range(8)))
    return gather_output(res.results, b_out)
